# revision 15
# baseline (speedup 1.0000x reference)
"""Trainium2 Bass kernel: conv2d(3x3,VALID) + bias -> min over C_out -> tanh(tanh).

Full-input contract: kernel(**inputs) takes the unsharded inputs
  x:           [32, 16, 256, 256] f32
  conv_weight: [64, 16, 3, 3]     f32
  conv_bias:   [64]               f32
and returns [32, 1, 254, 254] f32.

Strategy (data-parallel over batch, 4 images per core on 8 cores):
The conv is cast as matmuls over a kw-shifted slab in SBUF with J=3
position shifts per matmul column group (so only 5 kw taps -> 5x HBM
replication instead of 7x; the kernel is DMA-byte-bound at ~210 GB/s
per core since both NCs of an SEngine contend for the 16 SDMA ports):
  slab[kw*16+c, t] = x[c, t+kw]   (kw in 0..4), row 80 = ones (bias)
For block b (128*3 positions), tap kh:
  psum[m, (j,o)] += sum_k slab[k, 384b + 256kh + 3m] * W[o,c,kh,kw-j]
Channel-min is a free-dim reduce_min on DVE (batched 8 blocks/op),
then tanh(tanh()) on ACT, fp16 stores via SWDGE.

Residue column layout: within each window the slab columns are stored
grouped by (t mod 3), so every stationary operand is a CONTIGUOUS
128-column fp16 slice (stride-3 position steps become stride-1) ->
fast weight load; 256 = 3*85+1 keeps all three kh offsets contiguous.
Windows are 28/29 blocks so the per-window DMA (~9.5us) stays close to
per-window PE time (~7.2us) and PE idle gaps stay under the ~3.4us HAM
re-throttle threshold.
"""

import sys
import types

import numpy as np

# ---------------------------------------------------------------------------
# NTFF profile hook registration (the container's antenv stub lacks
# axon_hooks; registering it enables trace=True for profiling runs).
def _install_axon_hooks():
    try:
        import antenv.axon_hooks  # noqa: F401
        return
    except ImportError:
        pass
    try:
        import antenv
        from trn_agent_boot.trn_boot import _ntff_profile_via_ctypes
    except ImportError:
        return
    mod = types.ModuleType("antenv.axon_hooks")
    _hook = [None]
    mod.set_axon_ntff_profile_hook = lambda h: _hook.__setitem__(0, h)
    mod.get_axon_ntff_profile_hook = lambda: _hook[0]
    sys.modules["antenv.axon_hooks"] = mod
    antenv.axon_hooks = mod
    try:
        mod.set_axon_ntff_profile_hook(
            _ntff_profile_via_ctypes("/opt/axon/libaxon_pjrt.so")
        )
    except Exception:
        pass


_install_axon_hooks()

import concourse.bass as bass  # noqa: E402
import concourse.tile as tile  # noqa: E402
from concourse import bacc, mybir  # noqa: E402
from concourse.bass_utils import run_bass_kernel_spmd  # noqa: E402

N_CORES = 8
IMGS_PER_CORE = 4
C_IN, H, W = 16, 256, 256
C_OUT = 64
OH = OW = 254

J = 3                  # position shifts per matmul column group
TAPS = 5               # kw taps present in the contraction (0..4) = J+2
ROWS = TAPS * C_IN + 1  # 81 partitions: 5 kw-shifts x 16 ch + ones row
KPART = 128            # contraction padded to 128 rows (81..127 are zeros;
                       # zero weights there) for full-width weight loads
NFREE = J * C_OUT      # 192 moving columns per kh tap
BLK = 128 * J          # 384 flat positions per block
IMG = H * W            # 65536
XPAD = 65808           # padded flat image columns (zeros past 65536)

WINS = [29, 29, 28, 28, 28, 28]          # blocks per window (170 total)
WBASE = [0]
for _n in WINS[:-1]:
    WBASE.append(WBASE[-1] + _n * BLK)   # window base positions
# residue group size: smallest M >= 128*(nwb-1)+298+128 with M % 8 == 3 so
# every stationary start (kh*(M+85)+128b elems) is 16B-aligned
WM = [{29: 3883, 28: 3755}[n] for n in WINS]
WCOLS = [3 * m for m in WM]              # window col extents
WOFF = [0]
for _c in WCOLS[:-1]:
    WOFF.append(WOFF[-1] + _c)           # col offsets in the packed x rows
XCOLS = WOFF[-1] + WCOLS[-1]             # 68340 packed cols per slab row
MAXC = max(WCOLS)                        # 11646
YW = [J * n for n in WINS]               # output cols per window (87/84)
YOFF = [0]
for _ywn in YW[:-1]:
    YOFF.append(YOFF[-1] + _ywn)
YCOLS = YOFF[-1] + YW[-1]                # 510 output cols per image


def _groups(nwb):
    out = []
    b = 0
    while b < nwb:
        nb = min(8, nwb - b)
        out.append((b, nb))
        b += nb
    return out


def _prep_inputs(x, conv_weight, conv_bias):
    """Host-side packing: residue-ordered fp16 slab and matmul weights.

    slab[i, kw*16+c, woff_w + rr*M_w + q] = x[i, c, wbase_w + 3q + rr + kw]
    (kw in 0..4, rr in 0..2), row 80 = ones.  The stationary for
    (window w, block b, tap kh) is then the contiguous 128-col slice
    starting at woff_w + kh*M_w + 128*b + 85*kh.
    """
    n = x.shape[0]
    xf = np.zeros((n, C_IN, XPAD), dtype=np.float16)
    xf[:, :, :IMG] = x.reshape(n, C_IN, IMG)
    slab = np.zeros((n, ROWS, XCOLS), dtype=np.float16)
    for w in range(6):
        m = WM[w]
        for kw in range(TAPS):
            for rr in range(3):
                s0 = WBASE[w] + rr + kw
                slab[:, kw * C_IN:(kw + 1) * C_IN,
                     WOFF[w] + rr * m:WOFF[w] + (rr + 1) * m] = (
                    xf[:, :, s0:s0 + 3 * m:3]
                )
    slab[:, ROWS - 1, :] = 1.0

    # wmov[kw*16+c, kh*192 + j*64 + o] = W[o, c, kh, kw-j] (0 if kw-j not 0..2)
    wm = np.zeros((KPART, 3, J, C_OUT), dtype=np.float32)
    for kh in range(3):
        for j in range(J):
            for kk in range(3):
                kw = j + kk
                wm[kw * C_IN:(kw + 1) * C_IN, kh, j, :] = (
                    conv_weight[:, :, kh, kk].T
                )
    wm[ROWS - 1, 0, :, :] = conv_bias[None, :]  # bias via ones row, kh=0 only
    wmov = wm.reshape(KPART, 3 * NFREE).astype(np.float16)
    return slab, wmov


def _build_program():
    nc = bacc.Bacc(
        "TRN2", target_bir_lowering=False, debug=False, num_devices=N_CORES
    )
    f16 = mybir.dt.float16
    f32 = mybir.dt.float32

    x_d = nc.dram_tensor(
        "x", [IMGS_PER_CORE, ROWS, XCOLS], f16, kind="ExternalInput"
    )
    w_d = nc.dram_tensor("w", [KPART, 3 * NFREE], f16, kind="ExternalInput")
    z_d = nc.dram_tensor("z", [KPART - ROWS, MAXC], f16, kind="ExternalInput")
    y_d = nc.dram_tensor(
        "y", [IMGS_PER_CORE, 128, YCOLS], f16, kind="ExternalOutput"
    )

    with tile.TileContext(nc) as tc:
        with (
            tc.tile_pool(name="wpool", bufs=1) as wpool,
            tc.tile_pool(name="slab", bufs=3) as slab_pool,
            tc.tile_pool(name="stage", bufs=3) as stage_pool,
            tc.tile_pool(name="outp", bufs=2) as out_pool,
            tc.tile_pool(name="psum", bufs=2, space="PSUM") as psum_pool,
        ):
            w_t = wpool.tile([KPART, 3 * NFREE], f16)
            nc.sync.dma_start(w_t[:], w_d[:])

            # Zero the pad rows (81..127) of all three slab buffers once;
            # window DMAs only rewrite rows 0..80 so the zeros persist and
            # garbage (NaN) can never reach the matmul contraction.
            # (memset is not supported by this lowering path -> DMA zeros.)
            for _ in range(3):
                zt = slab_pool.tile([KPART, MAXC], f16, tag="slab")
                nc.sync.dma_start(zt[ROWS:KPART, :], z_d[:])

            # Warm-up matmuls on the weight tile while window 0 streams in:
            # keeps the PE HAM activity window busy so the clock gate opens
            # (K=8/8) before the real matmul stream begins.
            ps0 = psum_pool.tile([128, 4, 512], f32, tag="ps")
            for r in range(24):
                nc.tensor.matmul(
                    ps0[:, r % 4, 0:512],
                    w_t[:, 0:128],
                    w_t[:, 64:576],
                    start=True,
                    stop=True,
                )

            windows = [
                (i, w) for i in range(IMGS_PER_CORE) for w in range(6)
            ]

            def load_window(idx):
                i, w = windows[idx]
                cw = WCOLS[w]
                t = slab_pool.tile([KPART, MAXC], f16, tag="slab")
                # 80-row spray across all 16 SDMA engines; the constant
                # ones row goes separately ([1,N] also sprays by column).
                nc.sync.dma_start(
                    t[0:80, 0:cw], x_d[i, 0:80, WOFF[w]:WOFF[w] + cw]
                )
                nc.sync.dma_start(
                    t[80:81, 0:cw], x_d[i, 80:81, WOFF[w]:WOFF[w] + cw]
                )
                return t

            slabs = {0: load_window(0), 1: load_window(1)}
            th = None
            for idx in range(len(windows)):
                if idx + 2 < len(windows):
                    slabs[idx + 2] = load_window(idx + 2)
                i, w = windows[idx]
                slab = slabs.pop(idx)
                nwb = WINS[w]
                m = WM[w]
                if w == 0:
                    th = out_pool.tile([128, YCOLS], f16)
                mn = stage_pool.tile([128, max(YW)], f32, tag="mn")
                for g0, nb in _groups(nwb):
                    ps = psum_pool.tile([128, 4, 512], f32, tag="ps")
                    for s in range(nb):
                        b = g0 + s
                        bank = s // 2
                        off = NFREE * (s % 2)
                        for kh in range(3):
                            q0 = kh * m + 128 * b + 85 * kh
                            nc.tensor.matmul(
                                ps[:, bank, off:off + NFREE],
                                slab[0:KPART, q0:q0 + 128],
                                w_t[:, kh * NFREE:(kh + 1) * NFREE],
                                start=(kh == 0),
                                stop=(kh == 2),
                            )
                    nfull = nb // 2
                    b3 = J * g0
                    nc.vector.tensor_reduce(
                        mn[:, b3:b3 + 6 * nfull].rearrange(
                            "p (b sj) -> p b sj", sj=6
                        ),
                        ps[:, 0:nfull, 0:2 * NFREE].rearrange(
                            "p b (sj o) -> p b sj o", o=C_OUT
                        ),
                        axis=mybir.AxisListType.X,
                        op=mybir.AluOpType.min,
                    )
                    if nb % 2:
                        nc.vector.tensor_reduce(
                            mn[:, b3 + 6 * nfull:b3 + 6 * nfull + J],
                            ps[:, nfull, 0:NFREE].rearrange(
                                "p (j o) -> p j o", o=C_OUT
                            ),
                            axis=mybir.AxisListType.X,
                            op=mybir.AluOpType.min,
                        )
                t1 = stage_pool.tile([128, max(YW)], f32, tag="t1")
                yw = YW[w]
                nc.scalar.activation(
                    t1[:, 0:yw], mn[:, 0:yw],
                    mybir.ActivationFunctionType.Tanh,
                )
                nc.scalar.activation(
                    th[:, YOFF[w]:YOFF[w] + yw], t1[:, 0:yw],
                    mybir.ActivationFunctionType.Tanh,
                )
                if w == 5:
                    # SWDGE queue keeps output stores off the Sync FIFO
                    # so they never delay the slab prefetch DMAs.
                    nc.gpsimd.dma_start(y_d[i], th)
    nc.compile()
    return nc


_NC_CACHE = []


def _get_nc():
    if not _NC_CACHE:
        _NC_CACHE.append(_build_program())
    return _NC_CACHE[0]


def kernel(x, conv_weight, conv_bias, _trace=False):
    x = np.asarray(x, dtype=np.float32)
    conv_weight = np.asarray(conv_weight, dtype=np.float32)
    conv_bias = np.asarray(conv_bias, dtype=np.float32)
    n = x.shape[0]
    assert n == N_CORES * IMGS_PER_CORE

    slab, wmov = _prep_inputs(x, conv_weight, conv_bias)
    nc = _get_nc()
    zeros = np.zeros((KPART - ROWS, MAXC), dtype=np.float16)
    in_maps = [
        {
            "x": np.ascontiguousarray(
                slab[c * IMGS_PER_CORE:(c + 1) * IMGS_PER_CORE]
            ),
            "w": wmov,
            "z": zeros,
        }
        for c in range(N_CORES)
    ]
    res = run_bass_kernel_spmd(
        nc, in_maps, core_ids=list(range(N_CORES)), trace=_trace
    )
    arr = np.concatenate([r["y"] for r in res.results], axis=0)  # [32,128,510]
    flat = np.empty((n, WBASE[-1] + WINS[-1] * BLK), dtype=np.float32)
    for w in range(6):
        nwb = WINS[w]
        seg = arr[:, :, YOFF[w]:YOFF[w] + YW[w]].astype(np.float32)
        seg = seg.reshape(n, 128, nwb, J).transpose(0, 2, 1, 3)
        flat[:, WBASE[w]:WBASE[w] + nwb * BLK] = seg.reshape(n, nwb * BLK)
    y = flat[:, :OH * W].reshape(n, 1, OH, W)[:, :, :, :OW]
    out = np.ascontiguousarray(y)
    if _trace:
        kernel._last_result = res
    return out


# revision 20
# speedup vs baseline: 1.3859x; 1.3859x over previous
"""Trainium2 Bass kernel: conv2d(3x3,VALID) + bias -> min over C_out -> tanh(tanh).

Full-input contract: kernel(**inputs) takes the unsharded inputs
  x:           [32, 16, 256, 256] f32
  conv_weight: [64, 16, 3, 3]     f32
  conv_bias:   [64]               f32
and returns [32, 1, 254, 254] f32.

Strategy (data-parallel over batch, 4 images per core on 8 cores):
The conv is cast as matmuls over a kw-shifted slab in SBUF with J=3
position shifts per matmul column group (so only 5 kw taps -> 5x HBM
replication instead of 7x; the kernel is DMA-byte-bound at ~210 GB/s
per core since both NCs of an SEngine contend for the 16 SDMA ports):
  slab[kw*16+c, t] = x[c, t+kw]   (kw in 0..4), row 80 = ones (bias)
For block b (128*3 positions), tap kh:
  psum[m, (j,o)] += sum_k slab[k, 384b + 256kh + 3m] * W[o,c,kh,kw-j]
Channel-min is a free-dim reduce_min on DVE (batched 8 blocks/op),
then tanh(tanh()) on ACT, fp16 stores via SWDGE.

Residue column layout: within each window the slab columns are stored
grouped by (t mod 3), so every stationary operand is a CONTIGUOUS
128-column fp16 slice (stride-3 position steps become stride-1) ->
fast weight load; 256 = 3*85+1 keeps all three kh offsets contiguous.
Windows are 28/29 blocks so the per-window DMA (~9.5us) stays close to
per-window PE time (~7.2us) and PE idle gaps stay under the ~3.4us HAM
re-throttle threshold.
"""

import sys
import types

import numpy as np

# ---------------------------------------------------------------------------
# NTFF profile hook registration (the container's antenv stub lacks
# axon_hooks; registering it enables trace=True for profiling runs).
def _install_axon_hooks():
    try:
        import antenv.axon_hooks  # noqa: F401
        return
    except ImportError:
        pass
    try:
        import antenv
        from trn_agent_boot.trn_boot import _ntff_profile_via_ctypes
    except ImportError:
        return
    mod = types.ModuleType("antenv.axon_hooks")
    _hook = [None]
    mod.set_axon_ntff_profile_hook = lambda h: _hook.__setitem__(0, h)
    mod.get_axon_ntff_profile_hook = lambda: _hook[0]
    sys.modules["antenv.axon_hooks"] = mod
    antenv.axon_hooks = mod
    try:
        mod.set_axon_ntff_profile_hook(
            _ntff_profile_via_ctypes("/opt/axon/libaxon_pjrt.so")
        )
    except Exception:
        pass


_install_axon_hooks()

import concourse.bass as bass  # noqa: E402
import concourse.tile as tile  # noqa: E402
from concourse import bacc, mybir  # noqa: E402
from concourse.bass_utils import run_bass_kernel_spmd  # noqa: E402

N_CORES = 8
IMGS_PER_CORE = 4
C_IN, H, W = 16, 256, 256
C_OUT = 64
OH = OW = 254

J = 3                  # position shifts per matmul column group
TAPS = 5               # kw taps present in the contraction (0..4) = J+2
ROWS = TAPS * C_IN + 1  # 81 partitions: 5 kw-shifts x 16 ch + ones row
KPART = 128            # contraction padded to 128 rows (81..127 are zeros;
                       # zero weights there) for full-width weight loads
NFREE = J * C_OUT      # 192 moving columns per kh tap
BLK = 128 * J          # 384 flat positions per block
IMG = H * W            # 65536
XPAD = 65808           # padded flat image columns (zeros past 65536)

WINS = [29, 29, 28, 28, 28, 28]          # blocks per window (170 total)
WBASE = [0]
for _n in WINS[:-1]:
    WBASE.append(WBASE[-1] + _n * BLK)   # window base positions
# residue group size: smallest M >= 128*(nwb-1)+298+128 with M % 8 == 3 so
# every stationary start (kh*(M+85)+128b elems) is 16B-aligned
WM = [{29: 3883, 28: 3755}[n] for n in WINS]
WCOLS = [3 * m for m in WM]              # window col extents
WOFF = [0]
for _c in WCOLS[:-1]:
    WOFF.append(WOFF[-1] + _c)           # col offsets in the packed x rows
XCOLS = WOFF[-1] + WCOLS[-1]             # 68340 packed cols per slab row
MAXC = max(WCOLS)                        # 11646
YW = [J * n for n in WINS]               # output cols per window (87/84)
YOFF = [0]
for _ywn in YW[:-1]:
    YOFF.append(YOFF[-1] + _ywn)
YCOLS = YOFF[-1] + YW[-1]                # 510 output cols per image


def _groups(nwb):
    out = []
    b = 0
    while b < nwb:
        nb = min(8, nwb - b)
        out.append((b, nb))
        b += nb
    return out


def _prep_inputs(x, conv_weight, conv_bias):
    """Host-side packing: residue-ordered fp16 slab and matmul weights.

    slab[i, kw*16+c, woff_w + rr*M_w + q] = x[i, c, wbase_w + 3q + rr + kw]
    (kw in 0..4, rr in 0..2), row 80 = ones.  The stationary for
    (window w, block b, tap kh) is then the contiguous 128-col slice
    starting at woff_w + kh*M_w + 128*b + 85*kh.
    """
    n = x.shape[0]
    xf = np.zeros((n, C_IN, XPAD), dtype=np.float16)
    xf[:, :, :IMG] = x.reshape(n, C_IN, IMG)
    slab = np.zeros((n, ROWS - 1, XCOLS), dtype=np.float16)
    for w in range(6):
        m = WM[w]
        for kw in range(TAPS):
            for rr in range(3):
                s0 = WBASE[w] + rr + kw
                slab[:, kw * C_IN:(kw + 1) * C_IN,
                     WOFF[w] + rr * m:WOFF[w] + (rr + 1) * m] = (
                    xf[:, :, s0:s0 + 3 * m:3]
                )

    # wmov[kw*16+c, kh*192 + j*64 + o] = W[o, c, kh, kw-j] (0 if kw-j not 0..2)
    wm = np.zeros((KPART, 3, J, C_OUT), dtype=np.float32)
    for kh in range(3):
        for j in range(J):
            for kk in range(3):
                kw = j + kk
                wm[kw * C_IN:(kw + 1) * C_IN, kh, j, :] = (
                    conv_weight[:, :, kh, kk].T
                )
    wm[ROWS - 1, 0, :, :] = conv_bias[None, :]  # bias via ones row, kh=0 only
    wmov = wm.reshape(KPART, 3 * NFREE).astype(np.float16)
    return slab, wmov


def _build_program():
    nc = bacc.Bacc(
        "TRN2", target_bir_lowering=False, debug=False, num_devices=N_CORES
    )
    f16 = mybir.dt.float16
    f32 = mybir.dt.float32

    x_d = nc.dram_tensor(
        "x", [IMGS_PER_CORE, ROWS - 1, XCOLS], f16, kind="ExternalInput"
    )
    w_d = nc.dram_tensor("w", [KPART, 3 * NFREE], f16, kind="ExternalInput")
    # Constant slab rows 80..127: row 80 = ones (bias row), rest zeros
    # (pad rows so the contraction is a full 128 partitions). Written once
    # per slab buffer; window DMAs only touch rows 0..79.
    c_d = nc.dram_tensor("c", [KPART - ROWS + 1, MAXC], f16,
                         kind="ExternalInput")
    y_d = nc.dram_tensor(
        "y", [IMGS_PER_CORE, 128, YCOLS], f16, kind="ExternalOutput"
    )

    with tile.TileContext(nc) as tc:
        with (
            tc.tile_pool(name="wpool", bufs=1) as wpool,
            tc.tile_pool(name="slab", bufs=3) as slab_pool,
            tc.tile_pool(name="stage", bufs=3) as stage_pool,
            tc.tile_pool(name="outp", bufs=2) as out_pool,
            tc.tile_pool(name="psum", bufs=2, space="PSUM") as psum_pool,
        ):
            w_t = wpool.tile([KPART, 3 * NFREE], f16)
            nc.sync.dma_start(w_t[:], w_d[:])

            # Fill rows 80..127 (ones row + zero pad rows) of all three slab
            # buffers once: a 48-row transfer sprays across the SDMA engines
            # (a [1, N] ones row each window would pile onto one engine) and
            # garbage (NaN) can never reach the matmul contraction.
            for _ in range(3):
                zt = slab_pool.tile([KPART, MAXC], f16, tag="slab")
                nc.sync.dma_start(zt[ROWS - 1:KPART, :], c_d[:])

            # Warm-up matmuls on the weight tile while window 0 streams in:
            # keeps the PE HAM activity window busy so the clock gate opens
            # (K=8/8) before the real matmul stream begins.
            ps0 = psum_pool.tile([128, 4, 512], f32, tag="ps")
            for r in range(24):
                nc.tensor.matmul(
                    ps0[:, r % 4, 0:512],
                    w_t[:, 0:128],
                    w_t[:, 64:576],
                    start=True,
                    stop=True,
                )

            windows = [
                (i, w) for i in range(IMGS_PER_CORE) for w in range(6)
            ]

            def load_window(idx):
                i, w = windows[idx]
                cw = WCOLS[w]
                t = slab_pool.tile([KPART, MAXC], f16, tag="slab")
                # 80-row spray across all 16 SDMA engines.
                nc.sync.dma_start(
                    t[0:80, 0:cw], x_d[i, 0:80, WOFF[w]:WOFF[w] + cw]
                )
                return t

            slabs = {0: load_window(0), 1: load_window(1)}
            th = None
            for idx in range(len(windows)):
                if idx + 2 < len(windows):
                    slabs[idx + 2] = load_window(idx + 2)
                i, w = windows[idx]
                slab = slabs.pop(idx)
                nwb = WINS[w]
                m = WM[w]
                if w == 0:
                    th = out_pool.tile([128, YCOLS], f16)
                mn = stage_pool.tile([128, max(YW)], f32, tag="mn")
                for g0, nb in _groups(nwb):
                    ps = psum_pool.tile([128, 4, 512], f32, tag="ps")
                    for s in range(nb):
                        b = g0 + s
                        bank = s // 2
                        off = NFREE * (s % 2)
                        for kh in range(3):
                            q0 = kh * m + 128 * b + 85 * kh
                            nc.tensor.matmul(
                                ps[:, bank, off:off + NFREE],
                                slab[0:KPART, q0:q0 + 128],
                                w_t[:, kh * NFREE:(kh + 1) * NFREE],
                                start=(kh == 0),
                                stop=(kh == 2),
                            )
                    nfull = nb // 2
                    b3 = J * g0
                    nc.vector.tensor_reduce(
                        mn[:, b3:b3 + 6 * nfull].rearrange(
                            "p (b sj) -> p b sj", sj=6
                        ),
                        ps[:, 0:nfull, 0:2 * NFREE].rearrange(
                            "p b (sj o) -> p b sj o", o=C_OUT
                        ),
                        axis=mybir.AxisListType.X,
                        op=mybir.AluOpType.min,
                    )
                    if nb % 2:
                        nc.vector.tensor_reduce(
                            mn[:, b3 + 6 * nfull:b3 + 6 * nfull + J],
                            ps[:, nfull, 0:NFREE].rearrange(
                                "p (j o) -> p j o", o=C_OUT
                            ),
                            axis=mybir.AxisListType.X,
                            op=mybir.AluOpType.min,
                        )
                t1 = stage_pool.tile([128, max(YW)], f32, tag="t1")
                yw = YW[w]
                nc.scalar.activation(
                    t1[:, 0:yw], mn[:, 0:yw],
                    mybir.ActivationFunctionType.Tanh,
                )
                nc.scalar.activation(
                    th[:, YOFF[w]:YOFF[w] + yw], t1[:, 0:yw],
                    mybir.ActivationFunctionType.Tanh,
                )
                if w == 5:
                    # SWDGE queue keeps output stores off the Sync FIFO
                    # so they never delay the slab prefetch DMAs.
                    nc.gpsimd.dma_start(y_d[i], th)
    nc.compile()
    return nc


_NC_CACHE = []


def _get_nc():
    if not _NC_CACHE:
        _NC_CACHE.append(_build_program())
    return _NC_CACHE[0]


def kernel(x, conv_weight, conv_bias, _trace=False):
    x = np.asarray(x, dtype=np.float32)
    conv_weight = np.asarray(conv_weight, dtype=np.float32)
    conv_bias = np.asarray(conv_bias, dtype=np.float32)
    n = x.shape[0]
    assert n == N_CORES * IMGS_PER_CORE

    slab, wmov = _prep_inputs(x, conv_weight, conv_bias)
    nc = _get_nc()
    cpad = np.zeros((KPART - ROWS + 1, MAXC), dtype=np.float16)
    cpad[0, :] = 1.0
    in_maps = [
        {
            "x": np.ascontiguousarray(
                slab[c * IMGS_PER_CORE:(c + 1) * IMGS_PER_CORE]
            ),
            "w": wmov,
            "c": cpad,
        }
        for c in range(N_CORES)
    ]
    res = run_bass_kernel_spmd(
        nc, in_maps, core_ids=list(range(N_CORES)), trace=_trace
    )
    arr = np.concatenate([r["y"] for r in res.results], axis=0)  # [32,128,510]
    flat = np.empty((n, WBASE[-1] + WINS[-1] * BLK), dtype=np.float32)
    for w in range(6):
        nwb = WINS[w]
        seg = arr[:, :, YOFF[w]:YOFF[w] + YW[w]].astype(np.float32)
        seg = seg.reshape(n, 128, nwb, J).transpose(0, 2, 1, 3)
        flat[:, WBASE[w]:WBASE[w] + nwb * BLK] = seg.reshape(n, nwb * BLK)
    y = flat[:, :OH * W].reshape(n, 1, OH, W)[:, :, :, :OW]
    out = np.ascontiguousarray(y)
    if _trace:
        kernel._last_result = res
    return out


# revision 21
# speedup vs baseline: 1.7072x; 1.2318x over previous
"""Trainium2 Bass kernel: conv2d(3x3,VALID) + bias -> min over C_out -> tanh(tanh).

Full-input contract: kernel(**inputs) takes the unsharded inputs
  x:           [32, 16, 256, 256] f32
  conv_weight: [64, 16, 3, 3]     f32
  conv_bias:   [64]               f32
and returns [32, 1, 254, 254] f32.

Strategy (data-parallel over batch, 4 images per core on 8 cores):
conv as matmuls with J=4 position shifts per moving column group.  The key
byte-saving: with J=4 the stationary columns are flat positions t with
t % 4 == 0 (kh offsets are 256 = 4*64, so every kh tap stays on the same
residue), so the SBUF slab only stores every 4th image column:
  slab[kw*16+c, u] = x[c, 4u + kw]   (kw in 0..5)
which is only 1.5x the raw image bytes (the kernel was DMA-byte-bound at
~200 GB/s/core: both NCs of an SEngine share the 16 SDMA ports).
Row 96 = ones (bias via matmul), rows 97..127 = zeros: the contraction is
padded to the full 128 partitions (fast weight-path; zero rows cost no
time, LDWEIGHTS scales with columns).  Ones+zeros are written once per
slab buffer from a constant tensor; image DMAs only touch rows 0..95.

For block b (512 positions), tap kh (stationary = 128 contiguous cols at
u0 = 128b + 64kh, 16B-aligned):
  psum[m, (j,o)] += sum_k slab[k, u0 + m] * W[o, c, kh, kw-j]
Position p = 512b + 4m + j.  Channel-min is a free-dim reduce_min on DVE
batched 8 blocks (one 4-bank PSUM tile) per op, then tanh(tanh()) on ACT,
fp16 stores via SWDGE.  A short warm-up matmul burst at kernel start opens
the PE HAM clock gate (K=8/8) before the real stream begins.
"""

import sys
import types

import numpy as np

# ---------------------------------------------------------------------------
# NTFF profile hook registration (the container's antenv stub lacks
# axon_hooks; registering it enables trace=True for profiling runs).
def _install_axon_hooks():
    try:
        import antenv.axon_hooks  # noqa: F401
        return
    except ImportError:
        pass
    try:
        import antenv
        from trn_agent_boot.trn_boot import _ntff_profile_via_ctypes
    except ImportError:
        return
    mod = types.ModuleType("antenv.axon_hooks")
    _hook = [None]
    mod.set_axon_ntff_profile_hook = lambda h: _hook.__setitem__(0, h)
    mod.get_axon_ntff_profile_hook = lambda: _hook[0]
    sys.modules["antenv.axon_hooks"] = mod
    antenv.axon_hooks = mod
    try:
        mod.set_axon_ntff_profile_hook(
            _ntff_profile_via_ctypes("/opt/axon/libaxon_pjrt.so")
        )
    except Exception:
        pass


_install_axon_hooks()

import concourse.bass as bass  # noqa: E402
import concourse.tile as tile  # noqa: E402
from concourse import bacc, mybir  # noqa: E402
from concourse.bass_utils import run_bass_kernel_spmd  # noqa: E402

N_CORES = 8
IMGS_PER_CORE = 4
C_IN, H, W = 16, 256, 256
C_OUT = 64
OH = OW = 254

J = 4                  # position shifts per matmul column group
TAPS = 6               # kw taps in the contraction (0..5) = J+2
DROWS = TAPS * C_IN    # 96 data rows
KPART = 128            # contraction: 96 data + ones row 96 + zero pad
CROWS = KPART - DROWS  # 32 constant rows (ones + zeros)
NFREE = J * C_OUT      # 256 moving columns per kh tap
BLK = 128 * J          # 512 flat positions per block
IMG = H * W            # 65536
NB_IMG = 128           # blocks per image (covers all 65536 positions)
UCOLS = 128 * NB_IMG + 256  # 16640 stored columns (u = t//4), incl overhang
XPAD = 4 * UCOLS + TAPS     # padded flat image for host packing
GRP = 8                # blocks per PSUM tile / reduce op
YCOLS = NB_IMG * J     # 512 output cols per image: col = 4*b + j


def _prep_inputs(x, conv_weight, conv_bias):
    """Host-side packing: quarter-sampled fp16 slab and matmul weights.

    slab[i, kw*16+c, u] = x[i, c, 4u + kw]  (kw in 0..5, u in 0..UCOLS)
    wmov[kw*16+c, kh*256 + j*64 + o] = W[o, c, kh, kw-j] (0 outside 0..2),
    row 96 (ones) = bias at kh=0.
    """
    n = x.shape[0]
    xf = np.zeros((n, C_IN, XPAD), dtype=np.float16)
    xf[:, :, :IMG] = x.reshape(n, C_IN, IMG)
    slab = np.empty((n, DROWS, UCOLS), dtype=np.float16)
    for kw in range(TAPS):
        slab[:, kw * C_IN:(kw + 1) * C_IN, :] = (
            xf[:, :, kw:kw + 4 * UCOLS:4]
        )

    wm = np.zeros((KPART, 3, J, C_OUT), dtype=np.float32)
    for kh in range(3):
        for j in range(J):
            for kk in range(3):
                kw = j + kk
                wm[kw * C_IN:(kw + 1) * C_IN, kh, j, :] = (
                    conv_weight[:, :, kh, kk].T
                )
    wm[DROWS, 0, :, :] = conv_bias[None, :]  # bias via ones row, kh=0 only
    wmov = wm.reshape(KPART, 3 * NFREE).astype(np.float16)
    return slab, wmov


def _build_program():
    nc = bacc.Bacc(
        "TRN2", target_bir_lowering=False, debug=False, num_devices=N_CORES
    )
    f16 = mybir.dt.float16
    f32 = mybir.dt.float32

    x_d = nc.dram_tensor(
        "x", [IMGS_PER_CORE, DROWS, UCOLS], f16, kind="ExternalInput"
    )
    w_d = nc.dram_tensor("w", [KPART, 3 * NFREE], f16, kind="ExternalInput")
    # Constant slab rows 96..127: ones row (bias) then zero pad rows.
    c_d = nc.dram_tensor("c", [CROWS, UCOLS], f16, kind="ExternalInput")
    y_d = nc.dram_tensor(
        "y", [IMGS_PER_CORE, 128, YCOLS], f16, kind="ExternalOutput"
    )

    with tile.TileContext(nc) as tc:
        with (
            tc.tile_pool(name="wpool", bufs=1) as wpool,
            tc.tile_pool(name="slab", bufs=3) as slab_pool,
            tc.tile_pool(name="stage", bufs=3) as stage_pool,
            tc.tile_pool(name="outp", bufs=2) as out_pool,
            tc.tile_pool(name="psum", bufs=2, space="PSUM") as psum_pool,
        ):
            w_t = wpool.tile([KPART, 3 * NFREE], f16)
            nc.sync.dma_start(w_t[:], w_d[:])

            # Fill rows 96..127 (ones + zero pad) of all three slab buffers
            # once, on the SWDGE queue so it overlaps the first image loads;
            # image DMAs only rewrite rows 0..95 so the constants persist.
            for _ in range(3):
                zt = slab_pool.tile([KPART, UCOLS], f16, tag="slab")
                nc.gpsimd.dma_start(zt[DROWS:KPART, :], c_d[:])

            # Warm-up matmuls on the weight tile while image 0 streams in:
            # keeps the PE HAM activity window busy so the clock gate opens
            # (K=8/8) before the real matmul stream begins.
            ps0 = psum_pool.tile([128, 4, 512], f32, tag="ps")
            for r in range(24):
                nc.tensor.matmul(
                    ps0[:, r % 4, 0:512],
                    w_t[:, 0:128],
                    w_t[:, 256:768],
                    start=True,
                    stop=True,
                )

            def load_img(i):
                t = slab_pool.tile([KPART, UCOLS], f16, tag="slab")
                nc.sync.dma_start(t[0:DROWS, :], x_d[i])
                return t

            slabs = {0: load_img(0), 1: load_img(1)}
            for i in range(IMGS_PER_CORE):
                if i + 2 < IMGS_PER_CORE:
                    slabs[i + 2] = load_img(i + 2)
                slab = slabs.pop(i)
                mn = stage_pool.tile([128, YCOLS], f32, tag="mn")
                for g in range(NB_IMG // GRP):
                    ps = psum_pool.tile([128, 4, 512], f32, tag="ps")
                    for s in range(GRP):
                        b = g * GRP + s
                        bank = s // 2
                        off = NFREE * (s % 2)
                        for kh in range(3):
                            u0 = 128 * b + 64 * kh
                            nc.tensor.matmul(
                                ps[:, bank, off:off + NFREE],
                                slab[0:KPART, u0:u0 + 128],
                                w_t[:, kh * NFREE:(kh + 1) * NFREE],
                                start=(kh == 0),
                                stop=(kh == 2),
                            )
                    nc.vector.tensor_reduce(
                        mn[:, g * GRP * J:(g + 1) * GRP * J].rearrange(
                            "p (b sj) -> p b sj", sj=2 * J
                        ),
                        ps[:, :, :].rearrange(
                            "p b (sj o) -> p b sj o", o=C_OUT
                        ),
                        axis=mybir.AxisListType.X,
                        op=mybir.AluOpType.min,
                    )
                t1 = stage_pool.tile([128, YCOLS], f32, tag="t1")
                th = out_pool.tile([128, YCOLS], f16, tag="th")
                nc.scalar.activation(
                    t1[:], mn[:], mybir.ActivationFunctionType.Tanh,
                )
                nc.scalar.activation(
                    th[:], t1[:], mybir.ActivationFunctionType.Tanh,
                )
                # SWDGE queue keeps output stores off the Sync FIFO so they
                # never delay the slab prefetch DMAs.
                nc.gpsimd.dma_start(y_d[i], th)
    nc.compile()
    return nc


_NC_CACHE = []


def _get_nc():
    if not _NC_CACHE:
        _NC_CACHE.append(_build_program())
    return _NC_CACHE[0]


def kernel(x, conv_weight, conv_bias, _trace=False):
    x = np.asarray(x, dtype=np.float32)
    conv_weight = np.asarray(conv_weight, dtype=np.float32)
    conv_bias = np.asarray(conv_bias, dtype=np.float32)
    n = x.shape[0]
    assert n == N_CORES * IMGS_PER_CORE

    slab, wmov = _prep_inputs(x, conv_weight, conv_bias)
    nc = _get_nc()
    cpad = np.zeros((CROWS, UCOLS), dtype=np.float16)
    cpad[0, :] = 1.0
    in_maps = [
        {
            "x": np.ascontiguousarray(
                slab[c * IMGS_PER_CORE:(c + 1) * IMGS_PER_CORE]
            ),
            "w": wmov,
            "c": cpad,
        }
        for c in range(N_CORES)
    ]
    res = run_bass_kernel_spmd(
        nc, in_maps, core_ids=list(range(N_CORES)), trace=_trace
    )
    arr = np.concatenate([r["y"] for r in res.results], axis=0)  # [32,128,512]
    # col = 4*b + j, partition = m; position p = 512*b + 4*m + j
    seg = arr.astype(np.float32).reshape(n, 128, NB_IMG, J)
    flat = seg.transpose(0, 2, 1, 3).reshape(n, IMG)
    y = flat[:, :OH * W].reshape(n, 1, OH, W)[:, :, :, :OW]
    out = np.ascontiguousarray(y)
    if _trace:
        kernel._last_result = res
    return out


# revision 24
# speedup vs baseline: 1.8536x; 1.0858x over previous
"""Trainium2 Bass kernel: conv2d(3x3,VALID) + bias -> min over C_out -> tanh(tanh).

Full-input contract: kernel(**inputs) takes the unsharded inputs
  x:           [32, 16, 256, 256] f32
  conv_weight: [64, 16, 3, 3]     f32
  conv_bias:   [64]               f32
and returns [32, 1, 254, 254] f32.

Strategy (data-parallel over batch, 4 images per core on 8 cores):
conv as matmuls with J=4 position shifts per moving column group.  The key
byte-saving: with J=4 the stationary columns are flat positions t with
t % 4 == 0 (kh offsets are 256 = 4*64, so every kh tap stays on the same
residue), so the SBUF slab only stores every 4th image column:
  slab[kw*16+c, u] = x[c, 4u + kw]   (kw in 0..5)
which is only 1.5x the raw image bytes (the kernel was DMA-byte-bound at
~200 GB/s/core: both NCs of an SEngine share the 16 SDMA ports).
Row 96 = ones (bias via matmul), rows 97..127 = zeros: the contraction is
padded to the full 128 partitions (fast weight-path; zero rows cost no
time, LDWEIGHTS scales with columns).  Ones+zeros are written once per
slab buffer from a constant tensor; image DMAs only touch rows 0..95.

For block b (512 positions), tap kh (stationary = 128 contiguous cols at
u0 = 128b + 64kh, 16B-aligned):
  psum[m, (j,o)] += sum_k slab[k, u0 + m] * W[o, c, kh, kw-j]
Position p = 512b + 4m + j.  Channel-min is a free-dim reduce_min on DVE
batched 8 blocks (one 4-bank PSUM tile) per op, then tanh(tanh()) on ACT,
fp16 stores via SWDGE.  A short warm-up matmul burst at kernel start opens
the PE HAM clock gate (K=8/8) before the real stream begins.
"""

import sys
import types

import numpy as np

# ---------------------------------------------------------------------------
# NTFF profile hook registration (the container's antenv stub lacks
# axon_hooks; registering it enables trace=True for profiling runs).
def _install_axon_hooks():
    try:
        import antenv.axon_hooks  # noqa: F401
        return
    except ImportError:
        pass
    try:
        import antenv
        from trn_agent_boot.trn_boot import _ntff_profile_via_ctypes
    except ImportError:
        return
    mod = types.ModuleType("antenv.axon_hooks")
    _hook = [None]
    mod.set_axon_ntff_profile_hook = lambda h: _hook.__setitem__(0, h)
    mod.get_axon_ntff_profile_hook = lambda: _hook[0]
    sys.modules["antenv.axon_hooks"] = mod
    antenv.axon_hooks = mod
    try:
        mod.set_axon_ntff_profile_hook(
            _ntff_profile_via_ctypes("/opt/axon/libaxon_pjrt.so")
        )
    except Exception:
        pass


_install_axon_hooks()

import concourse.bass as bass  # noqa: E402
import concourse.tile as tile  # noqa: E402
from concourse import bacc, mybir  # noqa: E402
from concourse.bass_utils import run_bass_kernel_spmd  # noqa: E402

N_CORES = 8
IMGS_PER_CORE = 4
C_IN, H, W = 16, 256, 256
C_OUT = 64
OH = OW = 254

J = 4                  # position shifts per matmul column group
TAPS = 6               # kw taps in the contraction (0..5) = J+2
DROWS = TAPS * C_IN    # 96 data rows
KPART = 128            # contraction: 96 data + ones row 96 + zero pad
CROWS = KPART - DROWS  # 32 constant rows (ones + zeros)
NFREE = J * C_OUT      # 256 moving columns per kh tap
BLK = 128 * J          # 512 flat positions per block
IMG = H * W            # 65536
NB_IMG = 128           # blocks per image (covers all 65536 positions)
UCOLS = 128 * NB_IMG + 256  # 16640 stored columns (u = t//4), incl overhang
XPAD = 4 * UCOLS + TAPS     # padded flat image for host packing
GRP = 8                # blocks per PSUM tile / reduce op
YCOLS = NB_IMG * J     # 512 output cols per image: col = 4*b + j


def _prep_inputs(x, conv_weight, conv_bias):
    """Host-side packing: quarter-sampled fp16 slab and matmul weights.

    slab[i, kw*16+c, u] = x[i, c, 4u + kw]  (kw in 0..5, u in 0..UCOLS)
    wmov[kw*16+c, kh*256 + j*64 + o] = W[o, c, kh, kw-j] (0 outside 0..2),
    row 96 (ones) = bias at kh=0.
    """
    n = x.shape[0]
    xf = np.zeros((n, C_IN, XPAD), dtype=np.float16)
    xf[:, :, :IMG] = x.reshape(n, C_IN, IMG)
    slab = np.empty((n, DROWS, UCOLS), dtype=np.float16)
    for kw in range(TAPS):
        slab[:, kw * C_IN:(kw + 1) * C_IN, :] = (
            xf[:, :, kw:kw + 4 * UCOLS:4]
        )

    wm = np.zeros((KPART, 3, J, C_OUT), dtype=np.float32)
    for kh in range(3):
        for j in range(J):
            for kk in range(3):
                kw = j + kk
                wm[kw * C_IN:(kw + 1) * C_IN, kh, j, :] = (
                    conv_weight[:, :, kh, kk].T
                )
    wm[DROWS, 0, :, :] = conv_bias[None, :]  # bias via ones row, kh=0 only
    wmov = wm.reshape(KPART, 3 * NFREE).astype(np.float16)
    return slab, wmov


def _build_program():
    nc = bacc.Bacc(
        "TRN2", target_bir_lowering=False, debug=False, num_devices=N_CORES
    )
    f16 = mybir.dt.float16
    f32 = mybir.dt.float32

    x_d = nc.dram_tensor(
        "x", [IMGS_PER_CORE, DROWS, UCOLS], f16, kind="ExternalInput"
    )
    w_d = nc.dram_tensor("w", [KPART, 3 * NFREE], f16, kind="ExternalInput")
    # Constant slab rows 96..127: ones row (bias) then zero pad rows.
    c_d = nc.dram_tensor("c", [CROWS, UCOLS], f16, kind="ExternalInput")
    y_d = nc.dram_tensor(
        "y", [IMGS_PER_CORE, 128, YCOLS], f16, kind="ExternalOutput"
    )

    with tile.TileContext(nc) as tc:
        with (
            tc.tile_pool(name="wpool", bufs=1) as wpool,
            tc.tile_pool(name="slab", bufs=3) as slab_pool,
            tc.tile_pool(name="stage", bufs=3) as stage_pool,
            tc.tile_pool(name="outp", bufs=2) as out_pool,
            tc.tile_pool(name="psum", bufs=2, space="PSUM") as psum_pool,
        ):
            w_t = wpool.tile([KPART, 3 * NFREE], f16)
            nc.sync.dma_start(w_t[:], w_d[:])

            # Each image tile carries its own constant rows 96..127 (ones
            # row for the bias + zero pad rows): writing them into the same
            # tile gives the matmuls a real dependency edge on the fill (a
            # cross-tile "persistent" fill raced and left garbage bias).
            # Image 0 is loaded in two halves so its first blocks are ready
            # after ~half the transfer.
            def load_img(i, split=False):
                t = slab_pool.tile([KPART, UCOLS], f16, tag="slab")
                nc.sync.dma_start(t[DROWS:KPART, :], c_d[:])
                if split:
                    half = UCOLS // 2
                    nc.sync.dma_start(t[0:DROWS, 0:half], x_d[i, :, 0:half])
                    nc.sync.dma_start(
                        t[0:DROWS, half:UCOLS], x_d[i, :, half:UCOLS]
                    )
                else:
                    nc.sync.dma_start(t[0:DROWS, :], x_d[i])
                return t

            slabs = {0: load_img(0, split=True), 1: load_img(1)}

            # Warm-up matmuls on the weight tile while image 0 streams in:
            # keeps the PE HAM activity window busy so the clock gate opens
            # (K=8/8) and stays open until the real matmul stream begins.
            ps0 = psum_pool.tile([128, 4, 512], f32, tag="ps")
            for r in range(60):
                nc.tensor.matmul(
                    ps0[:, r % 4, 0:512],
                    w_t[:, 0:128],
                    w_t[:, 256:768],
                    start=True,
                    stop=True,
                )
            for i in range(IMGS_PER_CORE):
                if i + 2 < IMGS_PER_CORE:
                    slabs[i + 2] = load_img(i + 2)
                slab = slabs.pop(i)
                mn = stage_pool.tile([128, YCOLS], f32, tag="mn")
                for g in range(NB_IMG // GRP):
                    ps = psum_pool.tile([128, 4, 512], f32, tag="ps")
                    for s in range(GRP):
                        b = g * GRP + s
                        bank = s // 2
                        off = NFREE * (s % 2)
                        for kh in range(3):
                            u0 = 128 * b + 64 * kh
                            nc.tensor.matmul(
                                ps[:, bank, off:off + NFREE],
                                slab[0:KPART, u0:u0 + 128],
                                w_t[:, kh * NFREE:(kh + 1) * NFREE],
                                start=(kh == 0),
                                stop=(kh == 2),
                            )
                    nc.vector.tensor_reduce(
                        mn[:, g * GRP * J:(g + 1) * GRP * J].rearrange(
                            "p (b sj) -> p b sj", sj=2 * J
                        ),
                        ps[:, :, :].rearrange(
                            "p b (sj o) -> p b sj o", o=C_OUT
                        ),
                        axis=mybir.AxisListType.X,
                        op=mybir.AluOpType.min,
                    )
                t1 = stage_pool.tile([128, YCOLS], f32, tag="t1")
                th = out_pool.tile([128, YCOLS], f16, tag="th")
                nc.scalar.activation(
                    t1[:], mn[:], mybir.ActivationFunctionType.Tanh,
                )
                nc.scalar.activation(
                    th[:], t1[:], mybir.ActivationFunctionType.Tanh,
                )
                # SWDGE queue keeps output stores off the Sync FIFO so they
                # never delay the slab prefetch DMAs.
                nc.gpsimd.dma_start(y_d[i], th)
    nc.compile()
    return nc


_NC_CACHE = []


def _get_nc():
    if not _NC_CACHE:
        _NC_CACHE.append(_build_program())
    return _NC_CACHE[0]


def kernel(x, conv_weight, conv_bias, _trace=False):
    x = np.asarray(x, dtype=np.float32)
    conv_weight = np.asarray(conv_weight, dtype=np.float32)
    conv_bias = np.asarray(conv_bias, dtype=np.float32)
    n = x.shape[0]
    assert n == N_CORES * IMGS_PER_CORE

    slab, wmov = _prep_inputs(x, conv_weight, conv_bias)
    nc = _get_nc()
    cpad = np.zeros((CROWS, UCOLS), dtype=np.float16)
    cpad[0, :] = 1.0
    in_maps = [
        {
            "x": np.ascontiguousarray(
                slab[c * IMGS_PER_CORE:(c + 1) * IMGS_PER_CORE]
            ),
            "w": wmov,
            "c": cpad,
        }
        for c in range(N_CORES)
    ]
    res = run_bass_kernel_spmd(
        nc, in_maps, core_ids=list(range(N_CORES)), trace=_trace
    )
    arr = np.concatenate([r["y"] for r in res.results], axis=0)  # [32,128,512]
    # col = 4*b + j, partition = m; position p = 512*b + 4*m + j
    seg = arr.astype(np.float32).reshape(n, 128, NB_IMG, J)
    flat = seg.transpose(0, 2, 1, 3).reshape(n, IMG)
    y = flat[:, :OH * W].reshape(n, 1, OH, W)[:, :, :, :OW]
    out = np.ascontiguousarray(y)
    if _trace:
        kernel._last_result = res
    return out
